# revision 7
# baseline (speedup 1.0000x reference)
"""BitConv1d Trainium2 kernel (8 NeuronCores, data-parallel over batch).

Reference semantics (per batch b):
    x_n   = rmsnorm_over_C(x) * gamma
    scale = max(|x_n|) over the WHOLE tensor (global -> AllReduce max)
    n     = round(clip(x_n / scale * 127, -128, 127))        (integers in [-127,127])
    w_s   = max(mean(|w|), 1e-4)
    w_q   = round(clip(w / w_s, -1, 1))                      (ternary)
    out   = conv1d(n, w_q, pad=3) * (scale/127) * w_s

Key insight: n is an integer |n|<=127 (exact in bf16) and w_q is ternary
(exact in bf16), so the conv is EXACT integer arithmetic on the PE in
bf16 with fp32 PSUM accumulation.  All rounding is done with the fp32
magic-number trick (+1.5*2^23, RNE) which matches jnp.round.

Per core: batch b = core_id, x slice [512, 8192].
  Phase A: stream x, compute rms (ones-matmul partition-reduce broadcast),
           x_n -> DRAM scratch, local abs-max.
  AllReduce(max) of the local max (1 scalar).
  Weight quant + PE transposes overlap phase A / the collective.
  Phase B: quantize x_n -> bf16 n, 7x4x4 = 112 matmuls per 512-col chunk,
           scale PSUM by (w_s*scale/127), DMA out.
"""

import os
import sys
import types

import numpy as np


def _install_ntff_shim():
    """Make bass_utils' trace path work in containers lacking antenv.axon_hooks."""
    try:
        import antenv.axon_hooks  # noqa: F401
        return
    except ImportError:
        pass
    try:
        from trn_agent_boot.trn_boot import _ntff_profile_via_ctypes

        mod = types.ModuleType("antenv.axon_hooks")
        hook = _ntff_profile_via_ctypes("/opt/axon/libaxon_pjrt.so")
        mod.get_axon_ntff_profile_hook = lambda: hook
        mod.set_axon_ntff_profile_hook = lambda h: None
        sys.modules["antenv.axon_hooks"] = mod
        import antenv

        antenv.axon_hooks = mod
    except Exception:
        pass


_install_ntff_shim()

import concourse.bacc as bacc
import concourse.tile as tile
from concourse import mybir
from concourse.bass_utils import run_bass_kernel_spmd
from concourse.masks import make_identity

f32 = mybir.dt.float32
bf16 = mybir.dt.bfloat16

N_CORES = 8
C = 512          # in/out channels
T = 8192         # sequence length
KS = 7           # kernel taps
PAD = 3
NT = 4           # channel tiles of 128
CH = 512         # T-chunk width
NCH = T // CH    # 16
EPS = 1e-6
QP = 127.0
MAGIC = 12582912.0        # 1.5 * 2**23 : fp32 round-to-nearest-int magic
W_ELEMS = C * C * KS      # 1835008
HALO = CH + 2 * PAD       # 518


def _build(apply_gamma: bool, stage: int = 4):
    """stage (debug bisect): 1=phaseA, 2=+collective, 3=+weights, 4=full."""
    Alu = mybir.AluOpType
    ACTF = mybir.ActivationFunctionType

    nc = bacc.Bacc("TRN2", target_bir_lowering=False, debug=False,
                   num_devices=N_CORES)

    x_ext = nc.dram_tensor("x", [C, T], f32, kind="ExternalInput")
    w_ext = nc.dram_tensor("w", [C, C, KS], f32, kind="ExternalInput")
    nw_ext = nc.dram_tensor("nw", [C], f32, kind="ExternalInput")
    out_ext = nc.dram_tensor("out", [C, T], f32, kind="ExternalOutput")

    with tile.TileContext(nc) as tc:
        with (
            tc.tile_pool(name="consts", bufs=1) as consts,
            tc.tile_pool(name="wqt", bufs=1) as wqtp,
            tc.tile_pool(name="dram", bufs=1, space="DRAM") as dram,
        ):
            ones128 = consts.tile([128, 128], f32)
            nc.vector.memset(ones128[:], 1.0)
            ident = consts.tile([128, 128], f32)
            make_identity(nc, ident[:])
            eps_t = consts.tile([128, 1], f32)
            nc.vector.memset(eps_t[:], EPS)
            gamma = [consts.tile([128, 1], f32, name=f"gamma{j}") for j in range(NT)]
            for j in range(NT):
                nc.sync.dma_start(
                    out=gamma[j][:],
                    in_=nw_ext[j * 128:(j + 1) * 128].rearrange("(p o) -> p o", o=1),
                )
            mxbuf = consts.tile([128, NCH * NT], f32)
            wsums = consts.tile([128, NT], f32)
            # post-collective scalars
            sc128 = consts.tile([128, 1], f32)      # global act scale
            s127 = consts.tile([128, 1], f32)       # 127/scale
            gs = [consts.tile([128, 1], f32, name=f"gs{j}") for j in range(NT)]
            ws128 = consts.tile([128, 1], f32)      # weight scale
            osc = consts.tile([128, 1], f32)        # w_s*scale/127

            # ternary weights, bf16, lhsT layout [cin, cout], indexed (k, j, m)
            wqT = wqtp.tile([128, KS * NT * NT * 128], bf16)

            def wqT_sl(k, j, m):
                i = (k * NT + j) * NT + m
                return wqT[:, i * 128:(i + 1) * 128]

            xn_scr = dram.tile([NT, 128, T], f32)
            ccin = dram.tile([1, 1], f32)
            ccout = dram.tile([1, 1], f32)

            # ---------------- Phase A + weight prep + collective ------------
            with (
                tc.tile_pool(name="xin", bufs=8) as xinp,
                tc.tile_pool(name="sq", bufs=6) as sqp,
                tc.tile_pool(name="acc", bufs=2) as accp,
                tc.tile_pool(name="rms", bufs=2) as rmsp,
                tc.tile_pool(name="xn", bufs=8) as xnp,
                tc.tile_pool(name="wraw", bufs=4) as wrawp,
                tc.tile_pool(name="wsm", bufs=2) as wsmp,
                tc.tile_pool(name="psA", bufs=2, space="PSUM") as psA,
                tc.tile_pool(name="psT", bufs=2, space="PSUM") as psT,
                tc.tile_pool(name="psW", bufs=1, space="PSUM") as psW,
                tc.tile_pool(name="smal", bufs=2) as smal,
            ):
                # ---- phase A: rmsnorm + local abs-max, stream to scratch ----
                for ti in range(NCH):
                    t0 = ti * CH
                    xts, sqs = [], []
                    for j in range(NT):
                        xt = xinp.tile([128, CH], f32)
                        nc.sync.dma_start(
                            out=xt[:], in_=x_ext[j * 128:(j + 1) * 128, t0:t0 + CH])
                        sq = sqp.tile([128, CH], f32)
                        nc.scalar.square(sq[:], xt[:])
                        xts.append(xt)
                        sqs.append(sq)
                    s01 = accp.tile([128, CH], f32)
                    nc.vector.tensor_add(s01[:], sqs[0][:], sqs[1][:])
                    s23 = accp.tile([128, CH], f32)
                    nc.vector.tensor_add(s23[:], sqs[2][:], sqs[3][:])
                    x2s = accp.tile([128, CH], f32)
                    nc.vector.tensor_add(x2s[:], s01[:], s23[:])
                    # partition-reduce + broadcast: every partition gets sum_c x^2
                    ps = psA.tile([128, CH], f32)
                    nc.tensor.matmul(ps[:], ones128[:], x2s[:], start=True, stop=True)
                    sqv = rmsp.tile([128, CH], f32)
                    nc.scalar.activation(out=sqv[:], in_=ps[:], func=ACTF.Sqrt,
                                         bias=eps_t[:], scale=1.0 / C)
                    rms = rmsp.tile([128, CH], f32)
                    nc.vector.reciprocal(rms[:], sqv[:])
                    for j in range(NT):
                        xn = xnp.tile([128, CH], f32)
                        if apply_gamma:
                            nc.vector.scalar_tensor_tensor(
                                out=xn[:], in0=xts[j][:], scalar=gamma[j][:],
                                in1=rms[:], op0=Alu.mult, op1=Alu.mult)
                        else:
                            nc.vector.tensor_mul(xn[:], xts[j][:], rms[:])
                        nc.sync.dma_start(out=xn_scr[j, :, t0:t0 + CH], in_=xn[:])
                        nc.vector.tensor_reduce(
                            out=mxbuf[:, ti * NT + j: ti * NT + j + 1], in_=xn[:],
                            axis=mybir.AxisListType.X, op=Alu.max,
                            apply_absolute_value=True)

                if stage >= 2:
                    # ---- local max tree + AllReduce(max) ----
                    mx1 = smal.tile([128, 1], f32)
                    nc.vector.tensor_reduce(out=mx1[:], in_=mxbuf[:],
                                            axis=mybir.AxisListType.X, op=Alu.max)
                    mxt = smal.tile([1, 128], f32)
                    nc.sync.dma_start(out=mxt[:], in_=mx1[:])
                    mxs = smal.tile([1, 1], f32)
                    nc.vector.tensor_reduce(out=mxs[:], in_=mxt[:],
                                            axis=mybir.AxisListType.X, op=Alu.max)
                    mxc = smal.tile([1, 1], f32)
                    nc.vector.tensor_scalar_max(mxc[:], mxs[:], 1e-5)
                    nc.gpsimd.dma_start(out=ccin[:], in_=mxc[:])
                    nc.gpsimd.collective_compute(
                        "AllReduce", Alu.max,
                        replica_groups=[list(range(N_CORES))],
                        ins=[ccin.opt()], outs=[ccout.opt()],
                    )

                if stage >= 3:
                    # ---- weight quantization (overlaps phase A / collective) ----
                    wraws = []
                    for m in range(NT):
                        wraw = wrawp.tile([128, C * KS], f32)
                        nc.sync.dma_start(
                            out=wraw[:],
                            in_=w_ext[m * 128:(m + 1) * 128, :, :].rearrange(
                                "p c k -> p (c k)"))
                        wraws.append(wraw)
                        t56 = wsmp.tile([128, 56], f32)
                        nc.vector.tensor_reduce(
                            out=t56[:],
                            in_=wraw[:].rearrange("p (a b) -> p a b", b=64),
                            axis=mybir.AxisListType.X, op=Alu.add,
                            apply_absolute_value=True)
                        nc.vector.tensor_reduce(
                            out=wsums[:, m:m + 1], in_=t56[:],
                            axis=mybir.AxisListType.X, op=Alu.add)
                    wtot = wsmp.tile([128, 1], f32)
                    nc.vector.tensor_reduce(out=wtot[:], in_=wsums[:],
                                            axis=mybir.AxisListType.X, op=Alu.add)
                    pws = psW.tile([128, 1], f32)
                    nc.tensor.matmul(pws[:], ones128[:], wtot[:],
                                     start=True, stop=True)
                    wmean = wsmp.tile([128, 1], f32)
                    nc.scalar.activation(out=wmean[:], in_=pws[:], func=ACTF.Copy,
                                         scale=1.0 / W_ELEMS)
                    nc.vector.tensor_scalar_max(ws128[:], wmean[:], 1e-4)
                    winv = wsmp.tile([128, 1], f32)
                    nc.vector.reciprocal(winv[:], ws128[:])

                    for m in range(NT):
                        # in-place: wraw <- round(w/ws) + MAGIC, clip to MAGIC+-1
                        nc.scalar.activation(out=wraws[m][:], in_=wraws[m][:],
                                             func=ACTF.Copy, scale=winv[:],
                                             bias=MAGIC)
                        nc.vector.tensor_scalar(out=wraws[m][:], in0=wraws[m][:],
                                                scalar1=MAGIC + 1.0,
                                                scalar2=MAGIC - 1.0,
                                                op0=Alu.min, op1=Alu.max)
                        q3v = wraws[m][:].rearrange("p (c k) -> p c k", k=KS)
                        for k in range(KS):
                            for j in range(NT):
                                pst = psT.tile([128, 128], f32)
                                nc.tensor.matmul(pst[:],
                                                 q3v[:, j * 128:(j + 1) * 128, k],
                                                 ident[:], is_transpose=True)
                                # subtract MAGIC here -> ternary, bf16 exact
                                nc.scalar.activation(out=wqT_sl(k, j, m),
                                                     in_=pst[:], func=ACTF.Copy,
                                                     scale=1.0, bias=-MAGIC)

                if stage >= 2:
                    # ---- post-collective scalar setup ----
                    scg = smal.tile([1, 1], f32)
                    nc.gpsimd.dma_start(out=scg[:], in_=ccout[:])
                    nc.gpsimd.partition_broadcast(sc128[:], scg[:])
                    sinv = smal.tile([128, 1], f32)
                    nc.vector.reciprocal(sinv[:], sc128[:])
                    nc.vector.tensor_scalar_mul(s127[:], sinv[:], QP)
                    for j in range(NT):
                        if apply_gamma:
                            nc.vector.tensor_mul(gs[j][:], gamma[j][:], s127[:])
                        else:
                            nc.vector.tensor_copy(out=gs[j][:], in_=s127[:])
                    if stage >= 3:
                        nc.vector.tensor_mul(osc[:], ws128[:], sc128[:])
                        nc.vector.tensor_scalar_mul(osc[:], osc[:], 1.0 / QP)

            # ---------------- Phase B: quantize + conv matmuls ---------------
            if stage >= 4:
                with (
                    tc.tile_pool(name="xni", bufs=8) as xnip,
                    tc.tile_pool(name="qf", bufs=6) as qfp,
                    tc.tile_pool(name="nb", bufs=8) as nbp,
                    tc.tile_pool(name="ob", bufs=6) as obp,
                    tc.tile_pool(name="psC", bufs=6, space="PSUM") as psC,
                ):
                    for ti in range(NCH):
                        t0 = ti * CH
                        lo = max(t0 - PAD, 0)
                        hi = min(t0 + CH + PAD, T)
                        dst_lo = lo - (t0 - PAD)      # 3 for first chunk else 0
                        dst_hi = dst_lo + (hi - lo)
                        nbs = []
                        for j in range(NT):
                            xni = xnip.tile([128, HALO], f32)
                            if dst_lo > 0:
                                nc.vector.memset(xni[:, 0:dst_lo], 0.0)
                            if dst_hi < HALO:
                                nc.vector.memset(xni[:, dst_hi:HALO], 0.0)
                            nc.sync.dma_start(out=xni[:, dst_lo:dst_hi],
                                              in_=xn_scr[j, :, lo:hi])
                            qf = qfp.tile([128, HALO], f32)
                            nc.scalar.activation(out=qf[:], in_=xni[:],
                                                 func=ACTF.Copy,
                                                 scale=gs[j][:], bias=MAGIC)
                            # two copies: even-k taps read nb, odd-k taps read
                            # nb1 (shifted by 1) so every matmul rhs slice is
                            # 4-byte aligned (odd bf16 offsets fault the PE).
                            nb = nbp.tile([128, HALO], bf16)
                            nc.vector.tensor_scalar_sub(nb[:], qf[:], MAGIC)
                            nb1 = nbp.tile([128, HALO - 1], bf16)
                            nc.vector.tensor_scalar_sub(nb1[:], qf[:, 1:HALO],
                                                        MAGIC)
                            nbs.append((nb, nb1))
                        for m in range(NT):
                            pc = psC.tile([128, CH], f32)
                            idx = 0
                            for j in range(NT):
                                for k in range(KS):
                                    if k % 2 == 0:
                                        rhs = nbs[j][0][:, k:k + CH]
                                    else:
                                        rhs = nbs[j][1][:, k - 1:k - 1 + CH]
                                    nc.tensor.matmul(
                                        pc[:], wqT_sl(k, j, m), rhs,
                                        start=(idx == 0),
                                        stop=(idx == NT * KS - 1))
                                    idx += 1
                            ob = obp.tile([128, CH], f32)
                            nc.scalar.activation(out=ob[:], in_=pc[:],
                                                 func=ACTF.Copy, scale=osc[:])
                            nc.sync.dma_start(
                                out=out_ext[m * 128:(m + 1) * 128, t0:t0 + CH],
                                in_=ob[:])
            else:
                # debug: route xn scratch to the output so `out` is written
                with tc.tile_pool(name="dbg", bufs=4) as dbg:
                    for j in range(NT):
                        for ti in range(NCH):
                            t = dbg.tile([128, CH], f32)
                            nc.sync.dma_start(
                                out=t[:], in_=xn_scr[j, :, ti * CH:(ti + 1) * CH])
                            nc.sync.dma_start(
                                out=out_ext[j * 128:(j + 1) * 128,
                                            ti * CH:(ti + 1) * CH],
                                in_=t[:])

    nc.finalize()
    return nc


_NC_CACHE = {}


def _get_nc(apply_gamma: bool):
    stage = int(os.environ.get("BITCONV_STAGE", "4"))
    key = (apply_gamma, stage)
    if key not in _NC_CACHE:
        _NC_CACHE[key] = _build(apply_gamma, stage)
    return _NC_CACHE[key]


def _run(x, weight, norm_weight, trace=False, tmpdir=None):
    x = np.ascontiguousarray(x, dtype=np.float32)
    weight = np.ascontiguousarray(weight, dtype=np.float32)
    norm_weight = np.ascontiguousarray(norm_weight, dtype=np.float32)
    assert x.shape == (N_CORES, C, T), x.shape
    assert weight.shape == (C, C, KS), weight.shape
    assert norm_weight.shape == (C,), norm_weight.shape

    apply_gamma = not bool(np.all(norm_weight == np.float32(1.0)))
    nc = _get_nc(apply_gamma)
    in_maps = [
        {"x": x[i], "w": weight, "nw": norm_weight} for i in range(N_CORES)
    ]
    res = run_bass_kernel_spmd(nc, in_maps, list(range(N_CORES)),
                               trace=trace, tmpdir=tmpdir)
    out = np.stack([res.results[i]["out"] for i in range(N_CORES)], axis=0)
    return out, res.exec_time_ns


def kernel(x, weight, norm_weight):
    out, _ = _run(x, weight, norm_weight)
    return out
